# revision 3
# baseline (speedup 1.0000x reference)
"""AngleEmbedding kernel for 8 TRN2 NeuronCores.

The reference applies, per qubit q, the overwrite-semantics "rotation"
    new[i0] = 1j*sin(th/2)*state[i1];  new[i1] = cos(th/2)*state[i1]
(i1 = index with bit q set). Both outputs depend only on the bit=1
amplitudes. The initial state |0...0> has zero amplitude at every index
with any bit set, so the state is identically zero after the first
rotation and stays zero: the exact output is zeros((8, 2^20), complex64)
for every input x.

The kernel therefore reduces to materializing the 64 MiB zero output at
HBM write bandwidth. Sharding (per the state-vector-parallel hint): the
2^20 state axis is split across the 8 cores; each core owns 2^17 states
per batch row = 8 MiB of f32 (re,im) pairs, written by large HWDGE DMAs.

Per-core schedule (all zero-fill DMAs read one SBUF tile, so the write
stream never re-reads HBM):
  - gpsimd memsets a tiny [128, 256] head of the zero tile (its stream
    starts earliest after NEFF init) -> unlocks chunk 0 ASAP;
  - vector memsets the remaining [128, 256:2048] in parallel;
  - sync issues chunk 0 from the head via a step-0 repeat source AP,
    then chunks 1-3 from the full tile; scalar (the second HWDGE
    engine) issues chunks 4-7. The 16 SDMA engines stream the 8 MiB
    gap-free at ~405 GB/s per core (fabric ceiling is 435).
"""

import numpy as np

N_CORES = 8
BATCH = 8
N_QUBITS = 20
STATES = 1 << N_QUBITS                      # 1048576
SHARD_STATES = STATES // N_CORES            # 131072 states per core
SHARD_F32 = BATCH * SHARD_STATES * 2        # 2097152 f32 per core (8 MiB)
OUT_P = 128
OUT_F = SHARD_F32 // OUT_P                  # 16384
TILE_F = 2048                               # zero tile: [128, 2048] f32 = 1 MiB
M0_F = 256                                  # early head: [128, 256] f32

_CACHE = {}


def _build_nc():
    import concourse.bass as bass
    import concourse.mybir as mybir

    nc = bass.Bass()
    x = nc.declare_dram_parameter(
        "x", [BATCH, N_QUBITS], mybir.dt.float32, isOutput=False
    )
    out = nc.declare_dram_parameter(
        "out", [OUT_P, OUT_F], mybir.dt.float32, isOutput=True
    )

    chunk_f = TILE_F
    n_chunks = OUT_F // chunk_f             # 8 chunks x 1 MiB
    rep0 = chunk_f // M0_F

    def rep_ap(t, rep):
        # Read the tile `rep` times: partition dim first (must have nonzero
        # step), then a step-0 repeat dim over the per-partition run.
        return bass.AP(t.tensor, t.offset, [list(t.ap[0]), [0, rep], list(t.ap[1])])

    with (
        nc.sbuf_tensor([OUT_P, TILE_F], mybir.dt.float32) as ztile,
        nc.sbuf_tensor([BATCH, N_QUBITS], mybir.dt.float32) as xtile,
        nc.semaphore() as s0,
        nc.semaphore() as s1,
        nc.semaphore() as dsem,
        nc.semaphore() as xsem,
    ):
        t = ztile[:]
        t0 = ztile[:, :M0_F]
        # Emitted before the Block: these land right after NEFF init.
        nc.gpsimd.memset(t0, 0.0).then_inc(s0, 1)
        nc.vector.memset(ztile[:, M0_F:], 0.0).then_inc(s1, 1)

        def chunk_dst(k):
            return out[:, k * chunk_f:(k + 1) * chunk_f]

        with nc.Block() as block:
            @block.gpsimd
            def _(gpsimd):
                # Consume the angle input (the output is independent of it).
                gpsimd.dma_start(out=xtile[:], in_=x[:]).then_inc(xsem, 16)
                gpsimd.wait_ge(xsem, 16)

            @block.scalar
            def _(scalar):
                scalar.wait_ge(s1, 1)
                for k in range(n_chunks // 2, n_chunks):
                    scalar.dma_start(out=chunk_dst(k), in_=t).then_inc(dsem, 16)

            @block.sync
            def _(sync):
                sync.wait_ge(s0, 1)
                sync.dma_start(
                    out=chunk_dst(0).rearrange("p (r f) -> p r f", r=rep0),
                    in_=rep_ap(t0, rep0),
                ).then_inc(dsem, 16)
                sync.wait_ge(s1, 1)
                for k in range(1, n_chunks // 2):
                    sync.dma_start(out=chunk_dst(k), in_=t).then_inc(dsem, 16)
                sync.wait_ge(dsem, 16 * n_chunks)

    return nc


def _run(x, trace=False):
    from concourse.bass_utils import run_bass_kernel_spmd

    if "nc" not in _CACHE:
        _CACHE["nc"] = _build_nc()
    nc = _CACHE["nc"]

    xf = np.ascontiguousarray(np.asarray(x, dtype=np.float32))
    assert xf.shape == (BATCH, N_QUBITS)
    in_maps = [{"x": xf} for _ in range(N_CORES)]
    res = run_bass_kernel_spmd(
        nc, in_maps, core_ids=list(range(N_CORES)), trace=trace
    )
    # Core i holds states [i*SHARD_STATES, (i+1)*SHARD_STATES) for each
    # batch row, as interleaved (re, im) f32 pairs.
    parts = [
        res.results[i]["out"].reshape(BATCH, SHARD_STATES * 2)
        for i in range(N_CORES)
    ]
    full = np.ascontiguousarray(np.concatenate(parts, axis=1))
    return full.view(np.complex64), res


def kernel(x):
    out, _ = _run(x, trace=False)
    return out


# revision 5
# speedup vs baseline: 1.0516x; 1.0516x over previous
"""AngleEmbedding kernel for 8 TRN2 NeuronCores.

The reference applies, per qubit q, the overwrite-semantics "rotation"
    new[i0] = 1j*sin(th/2)*state[i1];  new[i1] = cos(th/2)*state[i1]
(i1 = index with bit q set). Both outputs depend only on the bit=1
amplitudes. The initial state |0...0> has zero amplitude at every index
with any bit set, so the state is identically zero after the first
rotation and stays zero: the exact output is zeros((8, 2^20), complex64)
for every input x.

The kernel therefore reduces to materializing the 64 MiB zero output at
HBM write bandwidth. Sharding (per the state-vector-parallel hint): the
2^20 state axis is split across the 8 cores; each core owns 2^17 states
per batch row = 8 MiB of f32 (re,im) pairs, written by large HWDGE DMAs.

Per-core schedule (all zero-fill DMAs read one SBUF tile, so the write
stream never re-reads HBM):
  - gpsimd memsets a tiny [128, 256] head of the zero tile (its stream
    starts earliest after NEFF init) -> unlocks chunk 0 ASAP;
  - vector memsets the remaining [128, 256:2048] in parallel;
  - sync issues chunk 0 from the head via a step-0 repeat source AP,
    then chunks 1-3 from the full tile; scalar (the second HWDGE
    engine) issues chunks 4-7. The 16 SDMA engines stream the 8 MiB
    gap-free at ~405 GB/s per core (fabric ceiling is 435).
"""

import numpy as np

N_CORES = 8
BATCH = 8
N_QUBITS = 20
STATES = 1 << N_QUBITS                      # 1048576
SHARD_STATES = STATES // N_CORES            # 131072 states per core
SHARD_F32 = BATCH * SHARD_STATES * 2        # 2097152 f32 per core (8 MiB)
OUT_P = 128
OUT_F = SHARD_F32 // OUT_P                  # 16384
TILE_F = 2048                               # zero tile: [128, 2048] f32 = 1 MiB
M0_F = 256                                  # early head: [128, 256] f32

_CACHE = {}


def _build_nc():
    import concourse.bass as bass
    import concourse.mybir as mybir

    nc = bass.Bass()
    x = nc.declare_dram_parameter(
        "x", [BATCH, N_QUBITS], mybir.dt.float32, isOutput=False
    )
    out = nc.declare_dram_parameter(
        "out", [OUT_P, OUT_F], mybir.dt.float32, isOutput=True
    )

    chunk_f = TILE_F
    n_chunks = OUT_F // chunk_f             # 8 chunks x 1 MiB
    rep0 = chunk_f // M0_F

    def rep_ap(t, rep):
        # Read the tile `rep` times: partition dim first (must have nonzero
        # step), then a step-0 repeat dim over the per-partition run.
        return bass.AP(t.tensor, t.offset, [list(t.ap[0]), [0, rep], list(t.ap[1])])

    with (
        nc.sbuf_tensor([OUT_P, TILE_F], mybir.dt.float32) as ztile,
        nc.sbuf_tensor([BATCH, N_QUBITS], mybir.dt.float32) as xtile,
        nc.semaphore() as s0,
        nc.semaphore() as s1,
        nc.semaphore() as dsem,
        nc.semaphore() as xsem,
    ):
        t = ztile[:]
        t0 = ztile[:, :M0_F]
        # Emitted before the Block: these land right after NEFF init.
        nc.gpsimd.memset(t0, 0.0).then_inc(s0, 1)
        nc.vector.memset(ztile[:, M0_F:], 0.0).then_inc(s1, 1)

        def chunk_dst(k):
            return out[:, k * chunk_f:(k + 1) * chunk_f]

        with nc.Block() as block:
            @block.gpsimd
            def _(gpsimd):
                # Consume the angle input (the output is independent of it).
                gpsimd.dma_start(out=xtile[:], in_=x[:]).then_inc(xsem, 16)
                gpsimd.wait_ge(xsem, 16)

            @block.scalar
            def _(scalar):
                # Chunks 4-7 read the whole tile: wait for both memset stages.
                scalar.wait_ge(s0, 1)
                scalar.wait_ge(s1, 1)
                for k in range(n_chunks // 2, n_chunks):
                    scalar.dma_start(out=chunk_dst(k), in_=t).then_inc(dsem, 16)

            @block.sync
            def _(sync):
                sync.wait_ge(s0, 1)
                sync.dma_start(
                    out=chunk_dst(0).rearrange("p (r f) -> p r f", r=rep0),
                    in_=rep_ap(t0, rep0),
                ).then_inc(dsem, 16)
                sync.wait_ge(s1, 1)
                for k in range(1, n_chunks // 2):
                    sync.dma_start(out=chunk_dst(k), in_=t).then_inc(dsem, 16)
                sync.wait_ge(dsem, 16 * n_chunks)

    return nc


def _run(x, trace=False):
    from concourse.bass_utils import run_bass_kernel_spmd

    if "nc" not in _CACHE:
        _CACHE["nc"] = _build_nc()
    nc = _CACHE["nc"]

    xf = np.ascontiguousarray(np.asarray(x, dtype=np.float32))
    assert xf.shape == (BATCH, N_QUBITS)
    in_maps = [{"x": xf} for _ in range(N_CORES)]
    try:
        res = run_bass_kernel_spmd(
            nc, in_maps, core_ids=list(range(N_CORES)), trace=trace
        )
    except Exception:
        # The axon-tunneled device occasionally throws a transient
        # NRT_EXEC_UNIT_UNRECOVERABLE; one retry clears it.
        res = run_bass_kernel_spmd(
            nc, in_maps, core_ids=list(range(N_CORES)), trace=trace
        )
    # Core i holds states [i*SHARD_STATES, (i+1)*SHARD_STATES) for each
    # batch row, as interleaved (re, im) f32 pairs.
    parts = [
        res.results[i]["out"].reshape(BATCH, SHARD_STATES * 2)
        for i in range(N_CORES)
    ]
    full = np.ascontiguousarray(np.concatenate(parts, axis=1))
    return full.view(np.complex64), res


def kernel(x):
    out, _ = _run(x, trace=False)
    return out
